# revision 3
# baseline (speedup 1.0000x reference)
"""Trainium2 Bass kernel for nn_Attention_86586540687646 — phase B.

Phase A data path (direct f32r DMA into compute tiles, [wq|wq]/[wk|wk]
column-duplicated projection weights so no partition-duplication DMAs,
host-duplicated 128-partition emb, DMAs spread over the SP/Pool/ACT
queues, software-prefetched next-iteration reloads), plus:

  - e_t and v_t in bf16 (post-softmax path only; q/k/logits stay f32r):
    halves their SBUF footprint to make room for double buffering. The
    fp32 PSUM accumulation is unchanged; end-to-end error ~1.7e-3.
  - q_s/kt_s/v_t double-buffered across repeat iterations, so iteration
    i+1's projections can run while iteration i's attention is still
    reading the other set.
  - iteration handoff: body i emits i+1's first q/k projection (during
    its h1 tail units) and i+1's first S^T+exp unit just before its own
    final AV, so the ACT engine rolls from i's last exp into i+1's first
    exp without waiting for i's drain. AV accumulators are allocated
    lazily (at first use in emit_av) to keep the 2-slot PSUM ring free of
    long-lived allocations at the boundary.

Sharding: 16 (batch, head) units across 8 cores -> core c handles batch
c//4 and heads (2*(c%4), 2*(c%4)+1), as before.
"""

import numpy as np

import concourse.mybir as mybir
import concourse.tile as tile
from concourse import bacc
from concourse.bass_utils import run_bass_kernel_spmd

B, C, H, W = 2, 512, 53, 53
HW = H * W            # 2809
NH, D = 8, 64
N_CORES = 8
HPC = 2               # heads per core
KO = C // 128         # 4 contraction chunks for the projection
JP = 2816             # j padded to 22*128
NJC = JP // 128       # 22 j-chunks
HWP = 2816            # i padded so every chunk is >=256 wide (fp32r full rate)
NIC = (HWP + 511) // 512  # 6 i-chunks (last one 256 wide)
JG = 3                # j-chunks per exp group
SHIFT = 54.0          # softmax logit shift (row maxes are in [26, 84])
SCALE = float(np.sqrt(D))

f32 = mybir.dt.float32
f32r = mybir.dt.float32r
bf16 = mybir.dt.bfloat16

_CACHE = {}

# j-group partition: NJC=22, JG=3 -> [3,3,3,3,3,3,2,2]
def _jgroups():
    sizes = []
    left = NJC
    while left > 0:
        sizes.append(min(JG, left))
        left -= sizes[-1]
    if len(sizes) >= 2 and sizes[-1] == 1:
        sizes[-1] = 2
        sizes[-2] -= 1
    out = []
    jc0 = 0
    for s in sizes:
        out.append((jc0, s))
        jc0 += s
    return out


def _emit_input_dmas_cold(nc, t, x_d, wqk_d, wv_d, emb_d):
    """First-iteration input loads, spread over the three DMA queues so the
    latency-critical pieces (x0, wqk, wv, emb head) land first."""
    nc.scalar.dma_start(t["wqk"][:], wqk_d.ap())
    nc.sync.dma_start(t["x_r"][:, :, 0:512], x_d.ap()[:, :, 0:512])
    nc.gpsimd.dma_start(t["wv"][:], wv_d.ap())
    nc.gpsimd.dma_start(t["emb"][:, 0:1024], emb_d.ap()[:, 0:1024])
    nc.gpsimd.dma_start(t["emb"][:, 1024:HWP], emb_d.ap()[:, 1024:HWP])
    for ic in range(1, NIC):
        i0 = ic * 512
        iw = min(512, HWP - i0)
        eng = nc.gpsimd if ic in (2, 4) else nc.sync
        eng.dma_start(t["x_r"][:, :, i0:i0 + iw], x_d.ap()[:, :, i0:i0 + iw])


def _emit_body(nc, tc, x_d, wqk_d, wv_d, emb_d, out_d, t, cur, nxt, pools,
               first, last, carry):
    Exp = mybir.ActivationFunctionType.Exp
    aps, vps, ep, npo = pools
    jgroups = _jgroups()
    NG = len(jgroups)
    x_r, wqk, wv, emb, nbias = (
        t["x_r"], t["wqk"], t["wv"], t["emb"], t["nbias"],
    )
    q_s, kt_s, v_t = cur["q_s"], cur["kt_s"], cur["v_t"]

    if first:
        _emit_input_dmas_cold(nc, t, x_d, wqk_d, wv_d, emb_d)
        nc.vector.memset(nbias[:], -SHIFT)
        # zero the j padding rows (121:128 of the last chunk) in BOTH
        # buffer sets; partition slices must be 32-aligned -> clear 96:128.
        for s_ in (cur, nxt):
            for h in range(HPC):
                nc.vector.memset(s_["v_t"][h][96:128, NJC - 1, :], 0.0)

    def prefetch_x(ic):
        """Emit next iteration's x-block DMA (after this body's last reader
        of that block)."""
        if last:
            return
        i0 = ic * 512
        iw = min(512, HWP - i0)
        eng = nc.gpsimd if ic in (2, 4) else nc.sync
        eng.dma_start(x_r[:, :, i0:i0 + iw], x_d.ap()[:, :, i0:i0 + iw])

    def proj_q(h, ic, psum_pool, tag, dst=cur):
        i0 = ic * 512
        iw = min(512, HWP - i0)
        psq = psum_pool.tile([128, 512], f32, tag=tag, name=f"pq{h}_{ic}")
        for ko in range(KO):
            nc.tensor.matmul(
                psq[:, :iw],
                wqk[:, ko, h * 256: h * 256 + 128],
                x_r[:, ko, i0:i0 + iw],
                start=(ko == 0), stop=(ko == KO - 1),
            )
        nc.vector.tensor_copy(dst["q_s"][h][:, i0:i0 + iw], psq[:, :iw])

    def proj_k(h, ic, psum_pool, tag, dst=cur):
        i0 = ic * 512
        iw = min(512, HWP - i0)
        psk = psum_pool.tile([128, 512], f32, tag=tag, name=f"pk{h}_{ic}")
        for ko in range(KO):
            nc.tensor.matmul(
                psk[:, :iw],
                wqk[:, ko, h * 256 + 128: h * 256 + 256],
                x_r[:, ko, i0:i0 + iw],
                start=(ko == 0), stop=(ko == KO - 1),
            )
        nc.vector.tensor_tensor(
            dst["kt_s"][h][:, i0:i0 + iw], psk[:, :iw], emb[:, i0:i0 + iw],
            mybir.AluOpType.add,
        )

    # ---- attention machinery (software pipeline over (h, ic, jgroup)
    # units; AV of unit u-1 is emitted after S/exp of unit u) ----
    units = [
        (h, ic, g)
        for h in range(HPC)
        for ic in range(NIC)
        for g in range(NG)
    ]
    avs = {}
    e_ts = {}
    if carry is not None:
        e_ts[0] = carry["e_t"]

    def emit_av(u):
        h, ic, g = units[u]
        i0 = ic * 512
        iw = min(512, HWP - i0)
        g0, gn = jgroups[g]
        e_t = e_ts.pop(u)
        if g == 0:
            # lazy allocation: the accumulator enters the 2-slot PSUM ring
            # only when its first AV matmul is emitted, so handoff-emitted
            # exp units don't pin a slot across the iteration boundary
            avs[(h, ic)] = vps.tile(
                [D + 1, 512], f32, tag="av", name=f"av_{h}_{ic}"
            )
        ps_av = avs[(h, ic)]
        for s in range(gn):
            jc = g0 + s
            nc.tensor.matmul(
                ps_av[:, :iw],
                v_t[h][:, jc, :],
                e_t[:, s, :iw],
                start=(jc == 0), stop=(jc == NJC - 1),
            )
        if g == NG - 1:
            acc = npo.tile([D + 1, 512], f32, tag="acc")
            nc.vector.tensor_copy(acc[:, :iw], ps_av[:, :iw])
            recip = npo.tile([1, 512], f32, tag="recip")
            nc.vector.reciprocal(recip[:, :iw], acc[D:D + 1, :iw])
            bcast = npo.tile([D, 512], f32, tag="bcast")
            nc.gpsimd.partition_broadcast(bcast[:, :iw], recip[:, :iw])
            o_s = npo.tile([D, 512], f32, tag="o")
            nc.vector.tensor_tensor(
                o_s[:, :iw], acc[0:D, :iw], bcast[:, :iw],
                mybir.AluOpType.mult,
            )
            ow = min(iw, HW - i0)
            nc.gpsimd.dma_start(
                out_d.ap()[h * D:(h + 1) * D, i0:i0 + ow], o_s[:, :ow]
            )
            del avs[(h, ic)]

    def emit_st_exp(u, qs, ks):
        """S^T matmuls + exp for unit u, reading the given q/kt set."""
        h, ic, g = units[u]
        i0 = ic * 512
        iw = min(512, HWP - i0)
        g0, gn = jgroups[g]
        ps_s = aps.tile([128, JG, 512], f32, tag="s")
        for s in range(gn):
            jc = g0 + s
            half = (jc % 2) * 64
            nc.tensor.matmul(
                ps_s[:, s, :iw],
                ks[h][half:half + 64, jc * 128:(jc + 1) * 128],
                qs[h][half:half + 64, i0:i0 + iw],
                start=True, stop=True,
                tile_position=(half, 0),
            )
        e_t = ep.tile([128, JG, 512], bf16, tag="e")
        nc.scalar.activation(
            e_t[:, :gn, :iw], ps_s[:, :gn, :iw], Exp,
            bias=nbias[:], scale=1.0,
        )
        return e_t

    def emit_unit(u):
        h, ic, g = units[u]
        e_ts[u] = emit_st_exp(u, q_s, kt_s)
        if u > 0:
            emit_av(u - 1)
        # head 1's projection rides in the PE slack of head 0's attention
        # phase; next-iteration reloads go out as their last readers retire;
        # the next iteration's first projection rides head 1's tail units
        if h == 0 and ic > 0 and g == NG - 2:
            proj_q(1, ic, vps, tag="av")
        elif h == 0 and ic > 0 and g == NG - 1:
            proj_k(1, ic, vps, tag="av")
            prefetch_x(ic)
            if ic == NIC - 1 and not last:
                nc.sync.dma_start(wqk[:], wqk_d.ap())
                nc.gpsimd.dma_start(emb[:], emb_d.ap())
        elif h == 1 and ic == NIC - 1 and g == NG - 2 and not last:
            # both at this hook: the two insertions keep the 2-slot "s"
            # ring's parity such that the handoff S^T tile below lands on a
            # slot freed well before this body's last exp
            proj_q(0, 0, aps, tag="s", dst=nxt)
            proj_k(0, 0, aps, tag="s", dst=nxt)

    blk_of_group = [
        ((jgroups[g][0] + jgroups[g][1]) * 128 - 1) // 512
        for g in range(NG)
    ]

    # ---- prologue: per 512-column block, project head 0 and V^T, and
    # start i-chunk-0 attention as soon as its j-dependencies land ----
    emitted = 1 if carry is not None else 0
    for ic in range(NIC):
        i0 = ic * 512
        iw = min(512, HWP - i0)
        if ic > 0 or carry is None:
            proj_q(0, ic, aps, tag="s")
            proj_k(0, ic, aps, tag="s")

        for jc in range(i0 // 128, min(NJC, (i0 + iw) // 128)):
            j0 = jc * 128
            jw = min(128, HW - j0)
            psv = vps.tile([128, 256], f32, tag="av", name=f"psv{jc}")
            for ko in range(KO):
                nc.tensor.matmul(
                    psv[:jw, :],
                    x_r[:, ko, j0:j0 + jw],
                    wv[:, ko, :],
                    start=(ko == 0), stop=(ko == KO - 1),
                )
            for h in range(HPC):
                nc.vector.tensor_copy(
                    v_t[h][:jw, jc, 0:D], psv[:jw, h * D:(h + 1) * D]
                )
                nc.vector.memset(v_t[h][:jw, jc, D:D + 1], 1.0)

        while emitted < NG and blk_of_group[emitted] <= ic:
            emit_unit(emitted)
            emitted += 1

    # wv's last reader is the prologue's psv matmuls: reload for the next
    # iteration now, ahead of the Pool queue's out DMAs
    if not last:
        nc.gpsimd.dma_start(wv[:], wv_d.ap())
    proj_q(1, 0, vps, tag="av")
    proj_k(1, 0, vps, tag="av")
    prefetch_x(0)
    for u in range(emitted, len(units)):
        emit_unit(u)

    # handoff: next iteration's first S^T+exp goes out BEFORE this body's
    # final AV, so ACT rolls straight from our last exp into the next
    # iteration's first exp (the S^T runs on PE during our last exp)
    out_carry = None
    if not last:
        out_carry = {"e_t": emit_st_exp(0, nxt["q_s"], nxt["kt_s"])}
    emit_av(len(units) - 1)
    return out_carry


def _alloc_set(pp, p):
    return {
        "q_s": [
            pp.tile([128, HWP], f32r, name=f"q_s{h}_{p}") for h in range(HPC)
        ],
        "kt_s": [
            pp.tile([128, JP], f32r, name=f"kt_s{h}_{p}") for h in range(HPC)
        ],
        "v_t": [
            pp.tile([128, NJC, D + 1], bf16, name=f"v_t{h}_{p}")
            for h in range(HPC)
        ],
    }


def build(repeats=1):
    nc = bacc.Bacc("TRN2", target_bir_lowering=False, debug=False)
    x_d = nc.dram_tensor("x", [128, KO, HWP], f32r, kind="ExternalInput")
    wqk_d = nc.dram_tensor(
        "wqk", [128, KO, 2 * HPC * 2 * D], f32r, kind="ExternalInput"
    )
    wv_d = nc.dram_tensor("wv", [128, KO, 256], f32r, kind="ExternalInput")
    emb_d = nc.dram_tensor("embT", [128, HWP], f32, kind="ExternalInput")
    out_d = nc.dram_tensor("out", [HPC * D, HW], f32, kind="ExternalOutput")
    with tile.TileContext(nc) as tc:
        with (
            tc.tile_pool(name="persist", bufs=1) as pp,
            tc.tile_pool(name="spsum", bufs=2, space="PSUM") as aps,
            tc.tile_pool(name="avpsum", bufs=2, space="PSUM") as vps,
            tc.tile_pool(name="epool", bufs=4) as ep,
            tc.tile_pool(name="npool", bufs=3) as npo,
        ):
            t = {
                "x_r": pp.tile([128, KO, HWP], f32r, name="x_r"),
                "emb": pp.tile([128, HWP], f32, name="emb"),
                "wqk": pp.tile(
                    [128, KO, 2 * HPC * 2 * D], f32r, name="wqk"
                ),
                "wv": pp.tile([128, KO, 256], f32r, name="wv"),
                "nbias": pp.tile([128, 1], f32, name="nbias"),
            }
            sets = [_alloc_set(pp, 0)]
            if repeats > 1:
                sets.append(_alloc_set(pp, 1))
            carry = None
            for i in range(repeats):
                carry = _emit_body(
                    nc, tc, x_d, wqk_d, wv_d, emb_d, out_d, t,
                    sets[i % len(sets)], sets[(i + 1) % len(sets)],
                    (aps, vps, ep, npo),
                    first=(i == 0), last=(i == repeats - 1),
                    carry=carry,
                )
    nc.compile()
    return nc


def make_in_maps(x, w_in, pos_h, pos_w):
    """Host-side sharding: per-core input dict."""
    x = np.ascontiguousarray(x, dtype=np.float32).reshape(B, C, HW)
    xp = np.zeros((B, C, HWP), dtype=np.float32)
    xp[:, :, :HW] = x
    w_in = np.asarray(w_in, dtype=np.float32)
    emb = (
        np.asarray(pos_h, np.float32)[:, None, :]
        + np.asarray(pos_w, np.float32)[None, :, :]
    ).reshape(HW, D)
    embT = np.zeros((D, HWP), dtype=np.float32)
    embT[:, :HW] = emb.T / SCALE
    emb128 = np.ascontiguousarray(np.tile(embT, (2, 1)))

    def lhsT(wrows):
        # (M, C) weight rows -> (128, KO, M) stationary layout
        return np.ascontiguousarray(
            wrows.T.reshape(KO, 128, wrows.shape[0]).transpose(1, 0, 2)
        )

    in_maps = []
    for c in range(N_CORES):
        b = c // (N_CORES // B)
        h0 = HPC * (c % (N_CORES // B))
        rows_qk = []
        rows_v = []
        for h in (h0, h0 + 1):
            wq = w_in[h * D:(h + 1) * D]
            wk = w_in[C + h * D: C + (h + 1) * D] * SCALE
            rows_qk += [wq, wq, wk, wk]                  # [q|q|k|k]
            rows_v.append(w_in[2 * C + h * D: 2 * C + (h + 1) * D])
        wv_rows = np.concatenate(
            rows_v + [np.zeros((256 - HPC * D, C), np.float32)], axis=0
        )
        xc = np.ascontiguousarray(
            xp[b].reshape(KO, 128, HWP).transpose(1, 0, 2)
        )
        in_maps.append({
            "x": xc,
            "wqk": lhsT(np.concatenate(rows_qk, axis=0)),
            "wv": lhsT(wv_rows),
            "embT": emb128,
        })
    return in_maps


def assemble(results):
    """Per-core (128, HW) slices -> (B, C, H, W)."""
    out = np.empty((B, C, HW), dtype=np.float32)
    for c in range(N_CORES):
        b = c // (N_CORES // B)
        h0 = HPC * (c % (N_CORES // B))
        out[b, h0 * D:(h0 + HPC) * D] = results[c]["out"]
    return out.reshape(B, C, H, W)


def kernel(x, w_in, pos_h, pos_w):
    if "nc" not in _CACHE:
        _CACHE["nc"] = build(repeats=1)
    nc = _CACHE["nc"]
    in_maps = make_in_maps(x, w_in, pos_h, pos_w)
    res = run_bass_kernel_spmd(nc, in_maps, core_ids=list(range(N_CORES)))
    return assemble(res.results)


# revision 4
# speedup vs baseline: 1.1729x; 1.1729x over previous
"""Trainium2 Bass kernel for nn_Attention_86586540687646 — phase B.

Phase A data path (direct f32r DMA into compute tiles, [wq|wq]/[wk|wk]
column-duplicated projection weights so no partition-duplication DMAs,
host-duplicated 128-partition emb, DMAs spread over the SP/Pool/ACT
queues, software-prefetched next-iteration reloads), plus:

  - e_t and v_t in bf16 (post-softmax path only; q/k/logits stay f32r):
    halves their SBUF footprint to make room for double buffering. The
    fp32 PSUM accumulation is unchanged; end-to-end error ~1.7e-3.
  - q_s/kt_s/v_t double-buffered across repeat iterations, so iteration
    i+1's projections can run while iteration i's attention is still
    reading the other set.
  - iteration handoff: body i emits i+1's first q/k projection (during
    its h1 tail units) and i+1's first S^T+exp unit just before its own
    final AV, so the ACT engine rolls from i's last exp into i+1's first
    exp without waiting for i's drain. AV accumulators are allocated
    lazily (at first use in emit_av) to keep the 2-slot PSUM ring free of
    long-lived allocations at the boundary.

Sharding: 16 (batch, head) units across 8 cores -> core c handles batch
c//4 and heads (2*(c%4), 2*(c%4)+1), as before.
"""

import numpy as np

import concourse.mybir as mybir
import concourse.tile as tile
from concourse import bacc
from concourse.bass_utils import run_bass_kernel_spmd

B, C, H, W = 2, 512, 53, 53
HW = H * W            # 2809
NH, D = 8, 64
N_CORES = 8
HPC = 2               # heads per core
KO = C // 128         # 4 contraction chunks for the projection
JP = 2816             # j padded to 22*128
NJC = JP // 128       # 22 j-chunks
HWP = 2816            # i padded so every chunk is >=256 wide (fp32r full rate)
NIC = (HWP + 511) // 512  # 6 i-chunks (last one 256 wide)
JG = 3                # j-chunks per exp group
SHIFT = 54.0          # softmax logit shift (row maxes are in [26, 84])
SCALE = float(np.sqrt(D))

f32 = mybir.dt.float32
f32r = mybir.dt.float32r
bf16 = mybir.dt.bfloat16

_CACHE = {}

# j-group partition: NJC=22, JG=3 -> [3,3,3,3,3,3,2,2]
def _jgroups():
    sizes = []
    left = NJC
    while left > 0:
        sizes.append(min(JG, left))
        left -= sizes[-1]
    if len(sizes) >= 2 and sizes[-1] == 1:
        sizes[-1] = 2
        sizes[-2] -= 1
    out = []
    jc0 = 0
    for s in sizes:
        out.append((jc0, s))
        jc0 += s
    return out


def _emit_input_dmas_cold(nc, t, x_d, wqk_d, wv_d, emb_d):
    """First-iteration input loads, spread over the three DMA queues so the
    latency-critical pieces (x0, wqk, wv, emb head) land first."""
    nc.scalar.dma_start(t["wqk"][:], wqk_d.ap())
    nc.sync.dma_start(t["x_r"][:, :, 0:512], x_d.ap()[:, :, 0:512])
    nc.gpsimd.dma_start(t["wv"][:], wv_d.ap())
    nc.gpsimd.dma_start(t["emb"][:, 0:1024], emb_d.ap()[:, 0:1024])
    nc.gpsimd.dma_start(t["emb"][:, 1024:HWP], emb_d.ap()[:, 1024:HWP])
    for ic in range(1, NIC):
        i0 = ic * 512
        iw = min(512, HWP - i0)
        eng = nc.gpsimd if ic in (2, 4) else nc.sync
        eng.dma_start(t["x_r"][:, :, i0:i0 + iw], x_d.ap()[:, :, i0:i0 + iw])


def _emit_body(nc, tc, x_d, wqk_d, wv_d, emb_d, out_d, t, cur, nxt, pools,
               first, last, carry):
    Exp = mybir.ActivationFunctionType.Exp
    aps, vps, ep, npo = pools
    jgroups = _jgroups()
    NG = len(jgroups)
    x_r, wqk, wv, emb, nbias = (
        t["x_r"], t["wqk"], t["wv"], t["emb"], t["nbias"],
    )
    q_s, kt_s, v_t = cur["q_s"], cur["kt_s"], cur["v_t"]

    if first:
        _emit_input_dmas_cold(nc, t, x_d, wqk_d, wv_d, emb_d)
        nc.vector.memset(nbias[:], -SHIFT)
        # zero the j padding rows (121:128 of the last chunk) in BOTH
        # buffer sets; partition slices must be 32-aligned -> clear 96:128.
        for s_ in (cur, nxt):
            for h in range(HPC):
                nc.vector.memset(s_["v_t"][h][96:128, NJC - 1, :], 0.0)

    def prefetch_x(ic):
        """Emit next iteration's x-block DMA (after this body's last reader
        of that block)."""
        if last:
            return
        i0 = ic * 512
        iw = min(512, HWP - i0)
        eng = nc.gpsimd if ic in (2, 4) else nc.sync
        eng.dma_start(x_r[:, :, i0:i0 + iw], x_d.ap()[:, :, i0:i0 + iw])

    def proj_q(h, ic, psum_pool, tag, dst=cur):
        i0 = ic * 512
        iw = min(512, HWP - i0)
        psq = psum_pool.tile([128, 512], f32, tag=tag, name=f"pq{h}_{ic}")
        for ko in range(KO):
            nc.tensor.matmul(
                psq[:, :iw],
                wqk[:, ko, h * 256: h * 256 + 128],
                x_r[:, ko, i0:i0 + iw],
                start=(ko == 0), stop=(ko == KO - 1),
            )
        nc.vector.tensor_copy(dst["q_s"][h][:, i0:i0 + iw], psq[:, :iw])

    def proj_k(h, ic, psum_pool, tag, dst=cur):
        i0 = ic * 512
        iw = min(512, HWP - i0)
        psk = psum_pool.tile([128, 512], f32, tag=tag, name=f"pk{h}_{ic}")
        for ko in range(KO):
            nc.tensor.matmul(
                psk[:, :iw],
                wqk[:, ko, h * 256 + 128: h * 256 + 256],
                x_r[:, ko, i0:i0 + iw],
                start=(ko == 0), stop=(ko == KO - 1),
            )
        nc.vector.tensor_tensor(
            dst["kt_s"][h][:, i0:i0 + iw], psk[:, :iw], emb[:, i0:i0 + iw],
            mybir.AluOpType.add,
        )

    # ---- attention machinery (software pipeline over (h, ic, jgroup)
    # units; AV of unit u-1 is emitted after S/exp of unit u) ----
    units = [
        (h, ic, g)
        for h in range(HPC)
        for ic in range(NIC)
        for g in range(NG)
    ]
    avs = {}
    e_ts = {}
    if carry is not None:
        e_ts[0] = carry["e_t"]

    def emit_av(u):
        h, ic, g = units[u]
        i0 = ic * 512
        iw = min(512, HWP - i0)
        g0, gn = jgroups[g]
        e_t = e_ts.pop(u)
        if g == 0:
            # lazy allocation: the accumulator enters the 2-slot PSUM ring
            # only when its first AV matmul is emitted, so handoff-emitted
            # exp units don't pin a slot across the iteration boundary
            avs[(h, ic)] = vps.tile(
                [D + 1, 512], f32, tag="av", name=f"av_{h}_{ic}"
            )
        ps_av = avs[(h, ic)]
        for s in range(gn):
            jc = g0 + s
            nc.tensor.matmul(
                ps_av[:, :iw],
                v_t[h][:, jc, :],
                e_t[:, s, :iw],
                start=(jc == 0), stop=(jc == NJC - 1),
            )
        if g == NG - 1:
            acc = npo.tile([D + 1, 512], f32, tag="acc")
            nc.vector.tensor_copy(acc[:, :iw], ps_av[:, :iw])
            recip = npo.tile([1, 512], f32, tag="recip")
            nc.vector.reciprocal(recip[:, :iw], acc[D:D + 1, :iw])
            bcast = npo.tile([D, 512], f32, tag="bcast")
            nc.gpsimd.partition_broadcast(bcast[:, :iw], recip[:, :iw])
            o_s = npo.tile([D, 512], f32, tag="o")
            nc.vector.tensor_tensor(
                o_s[:, :iw], acc[0:D, :iw], bcast[:, :iw],
                mybir.AluOpType.mult,
            )
            ow = min(iw, HW - i0)
            nc.gpsimd.dma_start(
                out_d.ap()[h * D:(h + 1) * D, i0:i0 + ow], o_s[:, :ow]
            )
            del avs[(h, ic)]

    def emit_st_exp(u, qs, ks):
        """S^T matmuls + exp for unit u, reading the given q/kt set."""
        h, ic, g = units[u]
        i0 = ic * 512
        iw = min(512, HWP - i0)
        g0, gn = jgroups[g]
        ps_s = aps.tile([128, JG, 512], f32, tag="s")
        for s in range(gn):
            jc = g0 + s
            half = (jc % 2) * 64
            nc.tensor.matmul(
                ps_s[:, s, :iw],
                ks[h][half:half + 64, jc * 128:(jc + 1) * 128],
                qs[h][half:half + 64, i0:i0 + iw],
                start=True, stop=True,
                tile_position=(half, 0),
            )
        e_t = ep.tile([128, JG, 512], bf16, tag="e")
        nc.scalar.activation(
            e_t[:, :gn, :iw], ps_s[:, :gn, :iw], Exp,
            bias=nbias[:], scale=1.0,
        )
        return e_t

    def emit_unit(u):
        h, ic, g = units[u]
        e_ts[u] = emit_st_exp(u, q_s, kt_s)
        if u > 0:
            emit_av(u - 1)
        # head 1's projection rides in the PE slack of head 0's attention
        # phase; next-iteration reloads go out as their last readers retire;
        # the next iteration's first projection rides head 1's tail units
        if h == 0 and ic > 0 and g == NG - 2:
            proj_q(1, ic, vps, tag="av")
        elif h == 0 and ic > 0 and g == NG - 1:
            proj_k(1, ic, vps, tag="av")
            prefetch_x(ic)
            if ic == NIC - 1 and not last:
                nc.sync.dma_start(wqk[:], wqk_d.ap())
                nc.gpsimd.dma_start(emb[:], emb_d.ap())
        elif h == 1 and ic == NIC - 1 and g == NG - 2 and not last:
            # both at this hook: the two insertions keep the 2-slot "s"
            # ring's parity such that the handoff S^T tile below lands on a
            # slot freed well before this body's last exp
            proj_q(0, 0, aps, tag="s", dst=nxt)
            proj_k(0, 0, aps, tag="s", dst=nxt)

    blk_of_group = [
        ((jgroups[g][0] + jgroups[g][1]) * 128 - 1) // 512
        for g in range(NG)
    ]

    # ---- prologue: per 512-column block, project head 0 and V^T, and
    # start i-chunk-0 attention as soon as its j-dependencies land.
    # In repeat iterations x is already resident (prefetched by the
    # previous body), so the q/k projections run one block AHEAD of the
    # V^T work: kt blocks then land at the exp pipeline's consumption
    # rate instead of being serialized behind each block's V matmuls. ----
    emitted = 1 if carry is not None else 0
    ahead = carry is not None
    if ahead:
        proj_q(0, 1, aps, tag="s")
        proj_k(0, 1, aps, tag="s")
    for ic in range(NIC):
        i0 = ic * 512
        iw = min(512, HWP - i0)
        if ahead:
            if ic + 2 < NIC:
                proj_q(0, ic + 2, aps, tag="s")
                proj_k(0, ic + 2, aps, tag="s")
        elif ic > 0 or carry is None:
            proj_q(0, ic, aps, tag="s")
            proj_k(0, ic, aps, tag="s")

        for jc in range(i0 // 128, min(NJC, (i0 + iw) // 128)):
            j0 = jc * 128
            jw = min(128, HW - j0)
            psv = vps.tile([128, 256], f32, tag="av", name=f"psv{jc}")
            for ko in range(KO):
                nc.tensor.matmul(
                    psv[:jw, :],
                    x_r[:, ko, j0:j0 + jw],
                    wv[:, ko, :],
                    start=(ko == 0), stop=(ko == KO - 1),
                )
            for h in range(HPC):
                nc.vector.tensor_copy(
                    v_t[h][:jw, jc, 0:D], psv[:jw, h * D:(h + 1) * D]
                )
                nc.vector.memset(v_t[h][:jw, jc, D:D + 1], 1.0)

        while emitted < NG and blk_of_group[emitted] <= ic:
            emit_unit(emitted)
            emitted += 1

    # wv's last reader is the prologue's psv matmuls: reload for the next
    # iteration now, ahead of the Pool queue's out DMAs
    if not last:
        nc.gpsimd.dma_start(wv[:], wv_d.ap())
    proj_q(1, 0, vps, tag="av")
    proj_k(1, 0, vps, tag="av")
    prefetch_x(0)
    for u in range(emitted, len(units)):
        emit_unit(u)

    # handoff: next iteration's first S^T+exp goes out BEFORE this body's
    # final AV, so ACT rolls straight from our last exp into the next
    # iteration's first exp (the S^T runs on PE during our last exp)
    out_carry = None
    if not last:
        out_carry = {"e_t": emit_st_exp(0, nxt["q_s"], nxt["kt_s"])}
    emit_av(len(units) - 1)
    return out_carry


def _alloc_set(pp, p):
    return {
        "q_s": [
            pp.tile([128, HWP], f32r, name=f"q_s{h}_{p}") for h in range(HPC)
        ],
        "kt_s": [
            pp.tile([128, JP], f32r, name=f"kt_s{h}_{p}") for h in range(HPC)
        ],
        "v_t": [
            pp.tile([128, NJC, D + 1], bf16, name=f"v_t{h}_{p}")
            for h in range(HPC)
        ],
    }


def build(repeats=1):
    nc = bacc.Bacc("TRN2", target_bir_lowering=False, debug=False)
    x_d = nc.dram_tensor("x", [128, KO, HWP], f32r, kind="ExternalInput")
    wqk_d = nc.dram_tensor(
        "wqk", [128, KO, 2 * HPC * 2 * D], f32r, kind="ExternalInput"
    )
    wv_d = nc.dram_tensor("wv", [128, KO, 256], f32r, kind="ExternalInput")
    emb_d = nc.dram_tensor("embT", [128, HWP], f32, kind="ExternalInput")
    out_d = nc.dram_tensor("out", [HPC * D, HW], f32, kind="ExternalOutput")
    with tile.TileContext(nc) as tc:
        with (
            tc.tile_pool(name="persist", bufs=1) as pp,
            tc.tile_pool(name="spsum", bufs=2, space="PSUM") as aps,
            tc.tile_pool(name="avpsum", bufs=2, space="PSUM") as vps,
            tc.tile_pool(name="epool", bufs=4) as ep,
            tc.tile_pool(name="npool", bufs=3) as npo,
        ):
            t = {
                "x_r": pp.tile([128, KO, HWP], f32r, name="x_r"),
                "emb": pp.tile([128, HWP], f32, name="emb"),
                "wqk": pp.tile(
                    [128, KO, 2 * HPC * 2 * D], f32r, name="wqk"
                ),
                "wv": pp.tile([128, KO, 256], f32r, name="wv"),
                "nbias": pp.tile([128, 1], f32, name="nbias"),
            }
            sets = [_alloc_set(pp, 0)]
            if repeats > 1:
                sets.append(_alloc_set(pp, 1))
            carry = None
            for i in range(repeats):
                carry = _emit_body(
                    nc, tc, x_d, wqk_d, wv_d, emb_d, out_d, t,
                    sets[i % len(sets)], sets[(i + 1) % len(sets)],
                    (aps, vps, ep, npo),
                    first=(i == 0), last=(i == repeats - 1),
                    carry=carry,
                )
    nc.compile()
    return nc


def make_in_maps(x, w_in, pos_h, pos_w):
    """Host-side sharding: per-core input dict."""
    x = np.ascontiguousarray(x, dtype=np.float32).reshape(B, C, HW)
    xp = np.zeros((B, C, HWP), dtype=np.float32)
    xp[:, :, :HW] = x
    w_in = np.asarray(w_in, dtype=np.float32)
    emb = (
        np.asarray(pos_h, np.float32)[:, None, :]
        + np.asarray(pos_w, np.float32)[None, :, :]
    ).reshape(HW, D)
    embT = np.zeros((D, HWP), dtype=np.float32)
    embT[:, :HW] = emb.T / SCALE
    emb128 = np.ascontiguousarray(np.tile(embT, (2, 1)))

    def lhsT(wrows):
        # (M, C) weight rows -> (128, KO, M) stationary layout
        return np.ascontiguousarray(
            wrows.T.reshape(KO, 128, wrows.shape[0]).transpose(1, 0, 2)
        )

    in_maps = []
    for c in range(N_CORES):
        b = c // (N_CORES // B)
        h0 = HPC * (c % (N_CORES // B))
        rows_qk = []
        rows_v = []
        for h in (h0, h0 + 1):
            wq = w_in[h * D:(h + 1) * D]
            wk = w_in[C + h * D: C + (h + 1) * D] * SCALE
            rows_qk += [wq, wq, wk, wk]                  # [q|q|k|k]
            rows_v.append(w_in[2 * C + h * D: 2 * C + (h + 1) * D])
        wv_rows = np.concatenate(
            rows_v + [np.zeros((256 - HPC * D, C), np.float32)], axis=0
        )
        xc = np.ascontiguousarray(
            xp[b].reshape(KO, 128, HWP).transpose(1, 0, 2)
        )
        in_maps.append({
            "x": xc,
            "wqk": lhsT(np.concatenate(rows_qk, axis=0)),
            "wv": lhsT(wv_rows),
            "embT": emb128,
        })
    return in_maps


def assemble(results):
    """Per-core (128, HW) slices -> (B, C, H, W)."""
    out = np.empty((B, C, HW), dtype=np.float32)
    for c in range(N_CORES):
        b = c // (N_CORES // B)
        h0 = HPC * (c % (N_CORES // B))
        out[b, h0 * D:(h0 + HPC) * D] = results[c]["out"]
    return out.reshape(B, C, H, W)


def kernel(x, w_in, pos_h, pos_w):
    if "nc" not in _CACHE:
        _CACHE["nc"] = build(repeats=1)
    nc = _CACHE["nc"]
    in_maps = make_in_maps(x, w_in, pos_h, pos_w)
    res = run_bass_kernel_spmd(nc, in_maps, core_ids=list(range(N_CORES)))
    return assemble(res.results)
